# revision 1
# baseline (speedup 1.0000x reference)
# Multi-head causal attention (B=4, T=2048, D=1024, H=16, dk=64), fp32.
#
# Sharding: 8 cores = 4 batches x 2 head-groups (8 heads / 512 cols each).
# Each core computes a partial output  y0_g @ wo_g  for its batch; the host
# sums the two head-group partials per batch and adds the constant row
# (bv @ wo + bo), which is exact because softmax rows sum to 1.
#
# Self-contained: hardcodes shapes; builds a Bass/Tile kernel and runs it on
# 8 NeuronCores via run_bass_kernel_spmd.

import numpy as np

B, T, D, H, DK = 4, 2048, 1024, 16, 64
NCORES = 8
G = 2               # head groups (tensor-parallel over heads)
C = D // G          # 512 columns per core = 8 heads
NH = C // DK        # heads per core = 8
NIB = T // 512      # 4 query blocks of 512
NJC = T // 128      # 16 key chunks of 128
SCALE = 1.0 / 8.0   # 1/sqrt(dk)

# Matmul dtype mode: "f32" (exact, 4 cyc/row) or "f32r" (fast, 1 cyc/row @ N>=256)
MM_MODE = "f32r"


def build_nc(mm_mode=MM_MODE, n_reps=1):
    from contextlib import ExitStack

    import concourse.bass as bass
    import concourse.mybir as mybir
    import concourse.tile as tile
    from concourse import bacc

    f32 = mybir.dt.float32
    u8 = mybir.dt.uint8
    AF = mybir.ActivationFunctionType
    mmdt = mybir.dt.float32r if mm_mode == "f32r" else None
    mdt = mybir.dt.float32r if mm_mode == "f32r" else f32
    def dsrc(ap):
        return ap.bitcast(mybir.dt.float32r) if mm_mode == "f32r" else ap

    nc = bacc.Bacc("TRN2", target_bir_lowering=False, debug=False,
                   num_devices=NCORES)

    x_d = nc.dram_tensor("x", [T, D], f32, kind="ExternalInput").ap()
    wq_d = nc.dram_tensor("wq", [D, C], f32, kind="ExternalInput").ap()
    wk_d = nc.dram_tensor("wk", [D, C], f32, kind="ExternalInput").ap()
    wv_d = nc.dram_tensor("wv", [D, C], f32, kind="ExternalInput").ap()
    wo_d = nc.dram_tensor("wo", [C, D], f32, kind="ExternalInput").ap()
    bq_d = nc.dram_tensor("bq", [C, 1], f32, kind="ExternalInput").ap()
    bk_d = nc.dram_tensor("bk", [C, 1], f32, kind="ExternalInput").ap()
    msk_d = nc.dram_tensor("invmask", [128, 128], f32, kind="ExternalInput").ap()
    vsel_d = nc.dram_tensor("vsel", [128, NJC, NH, NH], f32, kind="ExternalInput").ap()
    hsel_d = nc.dram_tensor("hsel", [72, NH, DK], f32, kind="ExternalInput").ap()
    idn_d = nc.dram_tensor("ident", [128, 128], f32, kind="ExternalInput").ap()
    out_d = nc.dram_tensor("out", [T, D], f32, kind="ExternalOutput").ap()

    with tile.TileContext(nc) as tc, ExitStack() as pers_stack:
        pers = pers_stack.enter_context(tc.tile_pool(name="pers", bufs=1))
        # transposed projections: qT/kT [128c(2 heads), T] per c-chunk
        qT = [pers.tile([128, T], mdt, name=f"qT{cc}", tag=f"qT{cc}")
              for cc in range(4)]
        kT = [pers.tile([128, T], mdt, name=f"kT{cc}", tag=f"kT{cc}")
              for cc in range(4)]
        # v in natural layout + one-hot denominator columns:
        # [j-in-chunk, chunk, head, dk+8]; cols 64..71 = onehot(head)
        v_ext = pers.tile([128, NJC, NH, DK + NH], mdt, name="v_ext", tag="v_ext")
        ident = pers.tile([128, 128], mdt, name="ident", tag="ident")
        hsel = pers.tile([72, NH, DK], mdt, name="hsel", tag="hsel")
        bq_sb = pers.tile([128, 4], f32, name="bq_sb", tag="bq_sb")
        bk_sb = pers.tile([128, 4], f32, name="bk_sb", tag="bk_sb")

        nc.sync.dma_start(ident[:], dsrc(idn_d)[:, :])
        nc.sync.dma_start(hsel[:], dsrc(hsel_d)[:, :, :])
        nc.sync.dma_start(v_ext[:, :, :, DK:DK + NH], dsrc(vsel_d)[:, :, :, :])
        for cc in range(4):
            nc.sync.dma_start(bq_sb[:, cc:cc + 1], bq_d[cc * 128:(cc + 1) * 128, :])
            nc.sync.dma_start(bk_sb[:, cc:cc + 1], bk_d[cc * 128:(cc + 1) * 128, :])

        for rep_ in range(n_reps):
            # ---------------- Stage A: projections ----------------
            with ExitStack() as sa:
                wpool = sa.enter_context(tc.tile_pool(name=f"wpool{rep_}", bufs=1))
                xpool = sa.enter_context(tc.tile_pool(name=f"xpool{rep_}", bufs=2))
                xTpool = sa.enter_context(tc.tile_pool(name=f"xTpool{rep_}", bufs=1))
                psA = sa.enter_context(tc.tile_pool(name=f"psA{rep_}", bufs=1, space="PSUM"))

                wq_sb = [wpool.tile([128, C], mdt, name=f"r{rep_}_wq{dc}", tag=f"wq{dc}")
                         for dc in range(8)]
                wk_sb = [wpool.tile([128, C], mdt, name=f"r{rep_}_wk{dc}", tag=f"wk{dc}")
                         for dc in range(8)]
                wv_sb = [wpool.tile([128, C], mdt, name=f"r{rep_}_wv{dc}", tag=f"wv{dc}")
                         for dc in range(8)]
                for dc in range(8):
                    nc.sync.dma_start(wq_sb[dc][:], dsrc(wq_d)[dc * 128:(dc + 1) * 128, :])
                    nc.sync.dma_start(wk_sb[dc][:], dsrc(wk_d)[dc * 128:(dc + 1) * 128, :])
                    nc.sync.dma_start(wv_sb[dc][:], dsrc(wv_d)[dc * 128:(dc + 1) * 128, :])

                for ib in range(NIB):
                    xT = [xTpool.tile([128, 512], mdt, name=f"r{rep_}_xT_{ib}_{dc}",
                                      tag=f"xT{dc}") for dc in range(8)]
                    for isub in range(4):
                        r0 = (ib * 4 + isub) * 128
                        xt = xpool.tile([128, D], mdt, name=f"r{rep_}_x_{ib}_{isub}", tag="x",
                                        bufs=4)
                        nc.sync.dma_start(xt[:], dsrc(x_d)[r0:r0 + 128, :])
                        for dc in range(8):
                            pt = psA.tile([128, 128], mdt, name=f"r{rep_}_pt_{ib}_{dc}_{isub}",
                                          tag="tr", bufs=3)
                            nc.tensor.transpose(
                                pt[:], xt[:, dc * 128:(dc + 1) * 128], ident[:])
                            nc.vector.tensor_copy(
                                xT[dc][:, isub * 128:(isub + 1) * 128], pt[:])
                    # qT / kT:  qT[c, i] = sum_d wq[d, c] * xT[d, i]  (+ bias)
                    for (wsb, dstT, bias_sb) in ((wq_sb, qT, bq_sb), (wk_sb, kT, bk_sb)):
                        for cc in range(4):
                            ps = psA.tile([128, 512], f32, name=f"r{rep_}_psq_{ib}_{cc}",
                                          tag="proj", bufs=3)
                            for dc in range(8):
                                nc.tensor.matmul(
                                    ps[:],
                                    (wsb[dc][:, cc * 128:(cc + 1) * 128]),
                                    (xT[dc][:]),
                                    start=(dc == 0), stop=(dc == 7))
                            nc.scalar.activation(
                                dstT[cc][:, ib * 512:(ib + 1) * 512], ps[:],
                                AF.Identity, bias=bias_sb[:, cc:cc + 1])
                    # v (no bias; folded into host constant): v[i, c]
                    for isub in range(4):
                        ps = psA.tile([128, C], f32, name=f"r{rep_}_psv_{ib}_{isub}",
                                      tag="proj", bufs=3)
                        for dc in range(8):
                            nc.tensor.matmul(
                                ps[:],
                                (xT[dc][:, isub * 128:(isub + 1) * 128]),
                                (wv_sb[dc][:]),
                                start=(dc == 0), stop=(dc == 7))
                        nc.vector.tensor_copy(
                            v_ext[:, ib * 4 + isub, :, 0:DK],
                            ps[:].rearrange("p (h d) -> p h d", d=DK))

            # ---------------- Stage B: attention + output projection ------------
            with ExitStack() as sb:
                wopool = sb.enter_context(tc.tile_pool(name=f"wopool{rep_}", bufs=1))
                mpool = sb.enter_context(tc.tile_pool(name=f"mpool{rep_}", bufs=1))
                epool = sb.enter_context(tc.tile_pool(name=f"epool{rep_}", bufs=4))
                ypool = sb.enter_context(tc.tile_pool(name=f"ypool{rep_}", bufs=9))
                npool = sb.enter_context(tc.tile_pool(name=f"npool{rep_}", bufs=2))
                opool = sb.enter_context(tc.tile_pool(name=f"opool{rep_}", bufs=2))
                psB = sb.enter_context(tc.tile_pool(name=f"psB{rep_}", bufs=1, space="PSUM"))

                wo_sb = [wopool.tile([128, D], mdt, name=f"r{rep_}_wo{cc}", tag=f"wo{cc}")
                         for cc in range(4)]
                for cc in range(4):
                    nc.sync.dma_start(wo_sb[cc][:], dsrc(wo_d)[cc * 128:(cc + 1) * 128, :])
                invm = mpool.tile([128, 128], f32, name=f"invm{rep_}", tag="invm")
                nc.sync.dma_start(invm[:], msk_d[:, :])

                for ib in range(NIB):
                    njc = 4 * (ib + 1)
                    yTu = [None] * NH
                    # per-head denominators accumulate (one-hot columns of
                    # v_ext put head h's denom in psum row 64+h, zeros else)
                    denacc = npool.tile([72, 512], f32, name=f"r{rep_}_den_{ib}",
                                        tag="den", bufs=2)
                    nc.vector.memset(denacc[DK:72, :], 0.0)
                    for hp in range(NH // 2):
                        h0, h1 = 2 * hp, 2 * hp + 1
                        psy0 = psB.tile([72, 512], f32, name=f"r{rep_}_psy_{ib}_{h0}",
                                        tag="y", bufs=2)
                        psy1 = psB.tile([72, 512], f32, name=f"r{rep_}_psy_{ib}_{h1}",
                                        tag="y", bufs=2)
                        for jc in range(njc):
                            # causal: diagonal chunks only need cols >= jc*128
                            o = max(0, jc - 4 * ib)
                            i0 = o * 128
                            nw = 512 - i0
                            # row-packed pair: heads h0/h1 run concurrently in
                            # PE row groups 0-1 / 2-3 (K=64 each)
                            pss0 = psB.tile([128, 512], f32,
                                            name=f"r{rep_}_pss_{ib}_{h0}_{jc}",
                                            tag="s", bufs=3)
                            pss1 = psB.tile([128, 512], f32,
                                            name=f"r{rep_}_pss_{ib}_{h1}_{jc}",
                                            tag="s", bufs=3)
                            nc.tensor.matmul(
                                pss0[:, 0:nw],
                                kT[hp][0:64, jc * 128:(jc + 1) * 128],
                                qT[hp][0:64, ib * 512 + i0:(ib + 1) * 512],
                                start=True, stop=True, tile_position=(0, 0))
                            nc.tensor.matmul(
                                pss1[:, 0:nw],
                                kT[hp][64:128, jc * 128:(jc + 1) * 128],
                                qT[hp][64:128, ib * 512 + i0:(ib + 1) * 512],
                                start=True, stop=True, tile_position=(64, 0))
                            if jc >= 4 * ib:
                                nc.vector.tensor_add(pss0[:, 0:128],
                                                     pss0[:, 0:128], invm[:])
                                nc.vector.tensor_add(pss1[:, 0:128],
                                                     pss1[:, 0:128], invm[:])
                            et0 = epool.tile([128, 512], mdt,
                                             name=f"r{rep_}_et_{ib}_{h0}_{jc}",
                                             tag="e", bufs=6)
                            et1 = epool.tile([128, 512], mdt,
                                             name=f"r{rep_}_et_{ib}_{h1}_{jc}",
                                             tag="e", bufs=6)
                            nc.scalar.activation(et0[:, 0:nw], pss0[:, 0:nw],
                                                 AF.Exp, scale=SCALE)
                            nc.scalar.activation(et1[:, 0:nw], pss1[:, 0:nw],
                                                 AF.Exp, scale=SCALE)
                            nc.tensor.matmul(
                                psy0[:, i0:512], v_ext[:, jc, h0, :],
                                et0[:, 0:nw],
                                start=(jc == 0), stop=(jc == njc - 1))
                            nc.tensor.matmul(
                                psy1[:, i0:512], v_ext[:, jc, h1, :],
                                et1[:, 0:nw],
                                start=(jc == 0), stop=(jc == njc - 1))
                        for h, psy in ((h0, psy0), (h1, psy1)):
                            yt = ypool.tile([DK, 512], f32,
                                            name=f"r{rep_}_yTu_{ib}_{h}",
                                            tag="yu", bufs=9)
                            nc.vector.tensor_copy(yt[:], psy[0:DK, :])
                            nc.vector.tensor_add(denacc[DK:72, :],
                                                 denacc[DK:72, :],
                                                 psy[DK:72, :])
                            yTu[h] = yt
                    rec = npool.tile([72, 512], f32, name=f"r{rep_}_rec_{ib}",
                                     tag="rec", bufs=2)
                    nc.vector.reciprocal(rec[DK:72, :], denacc[DK:72, :])
                    rec_r = npool.tile([72, 512], mdt, name=f"r{rep_}_recr_{ib}",
                                       tag="recr", bufs=2)
                    nc.vector.tensor_copy(rec_r[DK:72, :], rec[DK:72, :])
                    packed = [opool.tile([128, 512], mdt, name=f"r{rep_}_pk_{ib}_{cc}",
                                         tag=f"pk{cc}") for cc in range(4)]
                    for h in range(NH):
                        # broadcast recip row 64+h to 64 partitions via a K=8
                        # one-hot selector matmul (base partition 64 is legal)
                        pb = psB.tile([DK, 512], f32, name=f"r{rep_}_pb_{ib}_{h}",
                                      tag="bc", bufs=1)
                        nc.tensor.matmul(pb[:], hsel[DK:72, h, :],
                                         rec_r[DK:72, :], start=True, stop=True)
                        if h % 2 == 0:
                            nc.vector.tensor_mul(packed[h // 2][0:64, :],
                                                 yTu[h][:], pb[:])
                        else:
                            tmp = npool.tile([DK, 512], mdt,
                                             name=f"r{rep_}_tmp_{ib}_{h}",
                                             tag="tmp", bufs=2)
                            nc.vector.tensor_mul(tmp[:], yTu[h][:], pb[:])
                            nc.sync.dma_start(packed[h // 2][64:128, :], tmp[:])
                    # out[i, n] = sum_c yT[c, i] * wo[c, n]
                    for isub in range(4):
                        r0 = (ib * 4 + isub) * 128
                        osb = opool.tile([128, D], f32, name=f"r{rep_}_osb_{ib}_{isub}",
                                         tag="osb", bufs=2)
                        for nb in range(2):
                            pso = psB.tile([128, 512], f32, name=f"r{rep_}_pso_{ib}_{isub}_{nb}",
                                           tag="o", bufs=2)
                            for cc in range(4):
                                nc.tensor.matmul(
                                    pso[:],
                                    (packed[cc][:, isub * 128:(isub + 1) * 128]),
                                    (wo_sb[cc][:, nb * 512:(nb + 1) * 512]),
                                    start=(cc == 0), stop=(cc == 3))
                            if nb == 0:
                                nc.scalar.copy(osb[:, 0:512], pso[:])
                            else:
                                nc.vector.tensor_copy(osb[:, 512:1024], pso[:])
                        nc.sync.dma_start(out_d[r0:r0 + 128, :], osb[:])

    nc.compile()
    return nc


def make_in_maps(x, wq, bq, wk, bk, wv, bv, wo, bo):
    jj = np.arange(128)[:, None]
    ii = np.arange(128)[None, :]
    inv_masks = np.where(jj > ii, -1e9, 0.0).astype(np.float32)
    ident = np.eye(128, dtype=np.float32)
    eye8 = np.eye(8, dtype=np.float32)
    vsel = np.broadcast_to(eye8[None, None], (128, NJC, NH, NH)).copy()
    hsel = np.zeros((72, NH, DK), dtype=np.float32)
    hsel[DK:72] = eye8[:, :, None]

    in_maps = []
    for c in range(NCORES):
        b, g = c // G, c % G
        cs = slice(g * C, (g + 1) * C)
        in_maps.append({
            "x": np.ascontiguousarray(x[b]),
            "wq": np.ascontiguousarray(wq[:, cs]),
            "wk": np.ascontiguousarray(wk[:, cs]),
            "wv": np.ascontiguousarray(wv[:, cs]),
            "wo": np.ascontiguousarray(wo[cs, :]),
            "bq": np.ascontiguousarray(bq[cs].reshape(C, 1)),
            "bk": np.ascontiguousarray(bk[cs].reshape(C, 1)),
            "invmask": inv_masks,
            "ident": ident,
            "vsel": vsel,
            "hsel": hsel,
        })
    return in_maps


_NC_CACHE = {}


def _get_nc(mm_mode=MM_MODE):
    if mm_mode not in _NC_CACHE:
        _NC_CACHE[mm_mode] = build_nc(mm_mode)
    return _NC_CACHE[mm_mode]


def kernel(x, mask, wq, bq, wk, bk, wv, bv, wo, bo, _trace=False, _results=None):
    from concourse.bass_utils import run_bass_kernel_spmd

    x = np.asarray(x, dtype=np.float32)
    nc = _get_nc()
    in_maps = make_in_maps(x, np.asarray(wq), np.asarray(bq), np.asarray(wk),
                           np.asarray(bk), np.asarray(wv), np.asarray(bv),
                           np.asarray(wo), np.asarray(bo))
    res = run_bass_kernel_spmd(nc, in_maps, core_ids=list(range(NCORES)),
                               trace=_trace)
    if _results is not None:
        _results.append(res)
    # constant row: y += bv (since attn rows sum to 1)  =>  out += bv@wo + bo
    row_const = (np.asarray(bv, np.float64) @ np.asarray(wo, np.float64)
                 + np.asarray(bo, np.float64)).astype(np.float32)
    out = np.empty((B, T, D), dtype=np.float32)
    for b in range(B):
        out[b] = (res.results[2 * b]["out"] + res.results[2 * b + 1]["out"]
                  + row_const)
    return out



# revision 75
# speedup vs baseline: 37.2563x; 37.2563x over previous
# Multi-head causal attention (B=4, T=2048, D=1024, H=16, dk=64), fp32.
#
# Sharding: 8 cores = 4 batches x 2 head-groups (8 heads / 512 cols each).
# Each core computes a partial output  y0_g @ wo_g  for its batch; the host
# sums the two head-group partials per batch and adds the constant row
# (bv @ wo + bo), which is exact because softmax rows sum to 1.
#
# One global software pipeline over chunk steps (ib, hp, jc):
#   scores pair (PE, bf16) -> fused [128,2,512] exp (Act) -> attn@v pair (PE)
# with the causal mask pre-accumulated into PSUM by an identity matmul.
# All other work -- next block's projections (transpose + q/k/v matmuls),
# the previous block's output projection, and the softmax-denominator
# normalization -- is emitted as paced filler ops inside the chunk loop so
# the PE never drains (keeps DVFS p-state up) and the Act engine streams
# exps back to back.  DMAs are batched and split between the SP HWDGE queue
# (loads, via nc.sync) and the Pool SWDGE queue (stores, via nc.gpsimd);
# weights and x are cast to bf16 in-flight by the gpsimd DMA.
#
# Self-contained: hardcodes shapes; builds a Bass/Tile kernel and runs it on
# 8 NeuronCores via run_bass_kernel_spmd.

import numpy as np

B, T, D, H, DK = 4, 2048, 1024, 16, 64
NCORES = 8
G = 2               # head groups (tensor-parallel over heads)
C = D // G          # 512 columns per core = 8 heads
NH = C // DK        # heads per core = 8
NIB = T // 512      # 4 query blocks of 512
NJC = T // 128      # 16 key chunks of 128
SCALE = 1.0 / 8.0   # 1/sqrt(dk)

MM_MODE = "f32r"    # kept for test.py compatibility (attention runs bf16)


def build_nc(mm_mode=MM_MODE, n_reps=1, hw_loop=None, stages="AB", lead=2,
             pss_bufs=2, noexp=False, skip_outproj=False, sim_init=False,
             bias_on_act=False, vlag=6):
    from contextlib import ExitStack

    import concourse.bass as bass
    import concourse.mybir as mybir
    import concourse.tile as tile
    from concourse import bacc

    f32 = mybir.dt.float32
    bf16 = mybir.dt.bfloat16
    AF = mybir.ActivationFunctionType
    mdt = mybir.dt.float32r if mm_mode == "f32r" else f32

    nc = bacc.Bacc("TRN2", target_bir_lowering=False, debug=False,
                   num_devices=NCORES)

    x_d = nc.dram_tensor("x", [T, D], f32, kind="ExternalInput").ap()
    wq_d = nc.dram_tensor("wq", [D, C], f32, kind="ExternalInput").ap()
    wk_d = nc.dram_tensor("wk", [D, C], f32, kind="ExternalInput").ap()
    wv_d = nc.dram_tensor("wv", [D, C], f32, kind="ExternalInput").ap()
    wo_d = nc.dram_tensor("wo", [C, D], f32, kind="ExternalInput").ap()
    bq_d = nc.dram_tensor("bq", [C, 1], f32, kind="ExternalInput").ap()
    bk_d = nc.dram_tensor("bk", [C, 1], f32, kind="ExternalInput").ap()
    mskb_d = nc.dram_tensor("invmask_bf", [128, 128], bf16, kind="ExternalInput").ap()
    idnb_d = nc.dram_tensor("ident_bf", [128, 128], bf16, kind="ExternalInput").ap()
    idn_d = nc.dram_tensor("ident", [128, 128], f32, kind="ExternalInput").ap()
    out_d = nc.dram_tensor("out", [T, D], f32, kind="ExternalOutput").ap()

    with tile.TileContext(nc) as tc, ExitStack() as pers_stack:
        pers = pers_stack.enter_context(tc.tile_pool(name="pers", bufs=1))
        # transposed projections: qT/kT [128c(2 heads), T] per c-chunk, bf16
        qT = [pers.tile([128, T], bf16, name=f"qT{cc}", tag=f"qT{cc}")
              for cc in range(4)]
        kT = [pers.tile([128, T], bf16, name=f"kT{cc}", tag=f"kT{cc}")
              for cc in range(4)]
        # v in natural layout + an all-ones denominator column:
        # [j-in-chunk, chunk, head, dk+1]; col 64 = 1 => psy row 64 = sum(et)
        v_ext = pers.tile([128, NJC, NH, DK + 1], bf16, name="v_ext",
                          tag="v_ext")
        ident_bf = pers.tile([128, 128], bf16, name="ident_bf", tag="ident_bf")
        invm_bf = pers.tile([128, 128], bf16, name="invm_bf", tag="invm_bf")
        ident = pers.tile([128, 128], mdt, name="ident", tag="ident")
        bq_sb = pers.tile([128, 4], f32, name="bq_sb", tag="bq_sb")
        bk_sb = pers.tile([128, 4], f32, name="bk_sb", tag="bk_sb")
        # K=1 stationary ones row (at partition 64) for the recip broadcast
        ones_st = pers.tile([65, DK], bf16, name="ones_st", tag="ones_st")

        nc.sync.dma_start(ident_bf[:], idnb_d[:, :])
        nc.sync.dma_start(invm_bf[:], mskb_d[:, :])
        nc.sync.dma_start(ident[:], idn_d.bitcast(mybir.dt.float32r)[:, :]
                          if mm_mode == "f32r" else idn_d[:, :])
        nc.vector.memset(v_ext[:, :, :, DK:DK + 1], 1.0)
        nc.vector.memset(ones_st[DK:DK + 1, :], 1.0)
        for cc in range(4):
            nc.sync.dma_start(bq_sb[:, cc:cc + 1], bq_d[cc * 128:(cc + 1) * 128, :])
            nc.sync.dma_start(bk_sb[:, cc:cc + 1], bk_d[cc * 128:(cc + 1) * 128, :])
        if "A" not in stages:  # timing-only stage-B build: benign operands
            for cc in range(4):
                nc.vector.memset(qT[cc][:], 0.5)
                nc.vector.memset(kT[cc][:], 0.5)
            nc.vector.memset(v_ext[:, :, :, 0:DK], 0.5)

        loop_ctx = tc.For_i(0, hw_loop, 1) if hw_loop is not None else None
        if loop_ctx is not None:
            loop_ctx.__enter__()
        for rep_ in range(n_reps):
            with ExitStack() as sb:
                wpool = sb.enter_context(tc.tile_pool(name=f"wpool{rep_}", bufs=1))
                xpool = sb.enter_context(tc.tile_pool(name=f"xpool{rep_}", bufs=1))
                epool = sb.enter_context(tc.tile_pool(name=f"epool{rep_}", bufs=3))
                npool = sb.enter_context(tc.tile_pool(name=f"npool{rep_}", bufs=2))
                opool = sb.enter_context(tc.tile_pool(name=f"opool{rep_}", bufs=2))
                psB = sb.enter_context(tc.tile_pool(name=f"psB{rep_}", bufs=1,
                                                    space="PSUM"))

                do_A = "A" in stages
                do_B = "B" in stages or "C" in stages
                chunk_only = "C" in stages and "B" not in stages

                # -------- weights (bf16, one cast-DMA each via Pool SWDGE) --
                if do_A:
                    wq_sb = wpool.tile([128, 8, C], bf16, name=f"r{rep_}_wq", tag="wq")
                    wk_sb = wpool.tile([128, 8, C], bf16, name=f"r{rep_}_wk", tag="wk")
                    wv_sb = wpool.tile([128, 8, C], bf16, name=f"r{rep_}_wv", tag="wv")
                    for wsb, wd in ((wq_sb, wq_d), (wk_sb, wk_d), (wv_sb, wv_d)):
                        nc.gpsimd.dma_start(
                            wsb[:], wd.rearrange("(dc p) c -> p dc c", p=128))
                if not (chunk_only or skip_outproj):
                    wo_sb = wpool.tile([128, 4, D], bf16, name=f"r{rep_}_wo",
                                       tag="wo")
                    nc.gpsimd.dma_start(
                        wo_sb[:], wo_d.rearrange("(cc p) n -> p cc n", p=128))

                # -------- stage A emitters (as deferred closures) ----------
                xts = {}

                def emit_xdma(ib):
                    xt = xpool.tile([128, 4, D], mdt, name=f"r{rep_}_x_{ib}",
                                    tag="x")
                    nc.sync.dma_start(
                        xt[:],
                        x_d.bitcast(mybir.dt.float32r)[
                            ib * 512:(ib + 1) * 512, :].rearrange(
                            "(i p) d -> p i d", p=128))
                    xts[ib] = xt

                def proj_closures(ib):
                    """Transposes + q/k/v projections for query block ib."""
                    out = []
                    xT = xpool.tile([128, 8, 512], bf16, name=f"r{rep_}_xT_{ib}",
                                    tag="xT")
                    for isub in range(4):
                        for half in range(2):
                            def tr(isub=isub, half=half, ib=ib, xT=xT):
                                xt = xts[ib]
                                pt = psB.tile([128, 512], f32,
                                              name=f"r{rep_}_pt_{ib}_{isub}_{half}",
                                              tag="o", bufs=2)
                                for k in range(4):
                                    dc = half * 4 + k
                                    nc.tensor.transpose(
                                        pt[:, k * 128:(k + 1) * 128].bitcast(
                                            mybir.dt.float32r),
                                        xt[:, isub, dc * 128:(dc + 1) * 128],
                                        ident[:])
                                nc.vector.tensor_copy(
                                    xT[:, 4 * half:4 * half + 4,
                                       isub * 128:(isub + 1) * 128],
                                    pt[:].rearrange("p (k c) -> p k c", c=128))
                            out.append(tr)
                    for wsb, dstT, bias_sb in ((wq_sb, qT, bq_sb),
                                               (wk_sb, kT, bk_sb)):
                        for cc in range(4):
                            def qk(wsb=wsb, cc=cc, xT=xT, dstT=dstT,
                                   bias_sb=bias_sb, ib=ib):
                                ps = psB.tile(
                                    [128, 512], f32,
                                    name=f"r{rep_}_ps_{ib}_{id(wsb)}_{cc}",
                                    tag="o", bufs=2)
                                for dc in range(8):
                                    nc.tensor.matmul(
                                        ps[:],
                                        wsb[:, dc, cc * 128:(cc + 1) * 128],
                                        xT[:, dc, :],
                                        start=(dc == 0), stop=(dc == 7))
                                # bias on DVE keeps Act a pure exp stream
                                if bias_on_act:
                                    nc.scalar.activation(
                                        dstT[cc][:, ib * 512:(ib + 1) * 512],
                                        ps[:], AF.Identity,
                                        bias=bias_sb[:, cc:cc + 1])
                                else:
                                    nc.vector.tensor_scalar_add(
                                        dstT[cc][:, ib * 512:(ib + 1) * 512],
                                        ps[:], bias_sb[:, cc:cc + 1])
                            out.append(qk)
                    for isub in range(4):
                        def vproj(isub=isub, xT=xT, ib=ib):
                            ps = psB.tile([128, 512], f32,
                                          name=f"r{rep_}_psv_{ib}_{isub}",
                                          tag="o", bufs=2)
                            for dc in range(8):
                                nc.tensor.matmul(
                                    ps[:],
                                    xT[:, dc, isub * 128:(isub + 1) * 128],
                                    wv_sb[:, dc, :],
                                    start=(dc == 0), stop=(dc == 7))
                            nc.vector.tensor_copy(
                                v_ext[:, ib * 4 + isub, :, 0:DK],
                                ps[:].rearrange("p (h d) -> p h d", d=DK))
                        out.append(vproj)
                    return out

                # -------- stage B chunk pipeline ---------------------------
                steps = []
                for ib in range(NIB):
                    njc = 4 * (ib + 1)
                    for hp in range(NH // 2):
                        for jc in range(njc):
                            steps.append((ib, hp, jc))
                LEAD = lead
                psys = {}
                ets = {}
                packed = {}
                fillers = []   # (ready_step, fn)
                cur = {"t": 0}

                def push(delay, fn):
                    fillers.append((cur["t"] + delay, fn))

                def emit_S(t):
                    ib, hp, jc = steps[t]
                    o = max(0, jc - 4 * ib)
                    i0 = o * 128
                    nw = 512 - i0
                    diag = jc >= 4 * ib
                    pss = psB.tile([128, 2, 512], f32, name=f"r{rep_}_pss_{t}",
                                   tag="s", bufs=pss_bufs)
                    if sim_init and t < pss_bufs:
                        nc.vector.memset(pss[:], 0.0)
                    elif sim_init and nw < 512:
                        nc.vector.memset(pss[:, :, nw:512], 0.0)
                    q0 = ib * 512 + i0
                    for g in range(2):
                        ksl = kT[hp][64 * g:64 * g + 64,
                                     jc * 128:(jc + 1) * 128]
                        qsl = qT[hp][64 * g:64 * g + 64, :]
                        tp = (64 * g, 0)
                        if diag:
                            nc.tensor.matmul(pss[:, g, 0:128], ident_bf[:],
                                             invm_bf[:], start=True,
                                             stop=False)
                            nc.tensor.matmul(
                                pss[:, g, 0:128], ksl, qsl[:, q0:q0 + 128],
                                start=False, stop=True, tile_position=tp)
                            if nw > 128:
                                nc.tensor.matmul(
                                    pss[:, g, 128:nw], ksl,
                                    qsl[:, q0 + 128:(ib + 1) * 512],
                                    start=True, stop=True, tile_position=tp)
                        else:
                            nc.tensor.matmul(
                                pss[:, g, 0:nw], ksl,
                                qsl[:, q0:(ib + 1) * 512],
                                start=True, stop=True, tile_position=tp)
                    et = epool.tile([128, 2, 512], bf16, name=f"r{rep_}_et_{t}",
                                    tag="e", bufs=vlag + 1)
                    # full-width exp: stale cols are bounded scores (or -1e9
                    # masked -> 0), never inf; unused cols are never read
                    if noexp:
                        nc.vector.tensor_copy(et[:], pss[:])
                    else:
                        nc.scalar.activation(et[:], pss[:], AF.Exp, scale=SCALE)
                    ets[t] = et

                def emit_V(t):
                    ib, hp, jc = steps[t]
                    njc = 4 * (ib + 1)
                    o = max(0, jc - 4 * ib)
                    i0 = o * 128
                    nw = 512 - i0
                    if jc == 0:
                        psy0 = psB.tile([65, 512], f32, name=f"r{rep_}_psy_{t}_0",
                                        tag="y", bufs=2)
                        psy1 = psB.tile([65, 512], f32, name=f"r{rep_}_psy_{t}_1",
                                        tag="y", bufs=2)
                        psys[(ib, hp)] = (psy0, psy1)
                    psy0, psy1 = psys[(ib, hp)]
                    et = ets.pop(t)
                    nc.tensor.matmul(
                        psy0[:, i0:512], v_ext[:, jc, 2 * hp, :],
                        et[:, 0, 0:nw],
                        start=(jc == 0), stop=(jc == njc - 1))
                    nc.tensor.matmul(
                        psy1[:, i0:512], v_ext[:, jc, 2 * hp + 1, :],
                        et[:, 1, 0:nw],
                        start=(jc == 0), stop=(jc == njc - 1))
                    if jc == njc - 1 and not chunk_only:
                        emit_tail(ib, hp)

                def emit_tail(ib, hp):
                    psy0, psy1 = psys.pop((ib, hp))
                    if ib not in packed:
                        packed[ib] = [opool.tile([128, 512], bf16,
                                                 name=f"r{rep_}_pk_{ib}_{cc}",
                                                 tag=f"pk{cc}", bufs=2)
                                      for cc in range(4)]
                    pk = packed[ib][hp]
                    for h, psy in ((0, psy0), (1, psy1)):
                        # single copy releases the psum bank; recip/pb/mul
                        # run later off the critical path
                        yt = npool.tile([65, 512], f32,
                                        name=f"r{rep_}_yt_{ib}_{hp}_{h}",
                                        tag="yt", bufs=4)
                        nc.vector.tensor_copy(yt[:], psy[:])

                        def norm(h=h, yt=yt, pk=pk, ib=ib, hp=hp):
                            rec = npool.tile([65, 512], bf16,
                                             name=f"r{rep_}_rec_{ib}_{hp}_{h}",
                                             tag="rec", bufs=4)
                            with nc.allow_low_precision(
                                    reason="1/den bf16 feeds bf16 matmul"):
                                nc.vector.reciprocal(rec[DK:DK + 1, :],
                                                     yt[DK:DK + 1, :])
                            pb = psB.tile([128, 512], f32,
                                          name=f"r{rep_}_pb_{ib}_{hp}_{h}",
                                          tag="o", bufs=2)
                            nc.tensor.matmul(pb[0:64, :],
                                             ones_st[DK:DK + 1, :],
                                             rec[DK:DK + 1, :],
                                             start=True, stop=True)
                            if h == 0:
                                nc.vector.tensor_mul(pk[0:64, :], yt[0:DK, :],
                                                     pb[0:64, :])
                            else:
                                tmp = npool.tile([64, 512], bf16,
                                                 name=f"r{rep_}_tmp_{ib}_{hp}",
                                                 tag="tmp", bufs=2)
                                nc.vector.tensor_mul(tmp[:], yt[0:DK, :],
                                                     pb[0:64, :])
                                nc.gpsimd.dma_start(pk[64:128, :], tmp[:])
                        push(2 + 2 * h, norm)
                    if hp == NH // 2 - 1 and not skip_outproj:
                        queue_outproj(ib)

                def queue_outproj(ib):
                    pks = packed[ib]
                    for isub in range(4):
                        def filler(ib=ib, isub=isub, pks=pks):
                            osb = opool.tile([128, D], f32,
                                             name=f"r{rep_}_osb_{ib}_{isub}",
                                             tag="osb", bufs=2)
                            for nb in range(2):
                                pso = psB.tile(
                                    [128, 512], f32,
                                    name=f"r{rep_}_pso_{ib}_{isub}_{nb}",
                                    tag="o", bufs=2)
                                for cc in range(4):
                                    nc.tensor.matmul(
                                        pso[:],
                                        pks[cc][:, isub * 128:(isub + 1) * 128],
                                        wo_sb[:, cc, nb * 512:(nb + 1) * 512],
                                        start=(cc == 0), stop=(cc == 3))
                                nc.vector.tensor_copy(
                                    osb[:, nb * 512:(nb + 1) * 512], pso[:])
                            r0 = (ib * 4 + isub) * 128
                            nc.gpsimd.dma_start(out_d[r0:r0 + 128, :], osb[:])
                        push(4 + 2 * isub, filler)

                # -------- prologue: block 0 projections --------------------
                if do_A:
                    emit_xdma(0)
                    for fn in proj_closures(0):
                        fn()
                    if do_B:
                        emit_xdma(1)
                        pending_proj = proj_closures(1)
                    else:
                        for ib in range(1, NIB):
                            emit_xdma(ib)
                            for fn in proj_closures(ib):
                                fn()
                        pending_proj = []
                else:
                    pending_proj = []

                # -------- main pipelined loop ------------------------------
                if do_B:
                    nsteps = len(steps)
                    for t in range(nsteps + vlag):
                        cur["t"] = t
                        if t < nsteps:
                            ib = steps[t][0]
                            # drain next block's projections before its
                            # chunks start (PE in-order => emission order
                            # must respect the data dependency)
                            boundary = (t + 1 < nsteps
                                        and steps[t + 1][0] != ib)
                            if do_A and pending_proj:
                                left = sum(1 for s in steps[t:] if s[0] == ib)
                                want = 1 if left * 2 > len(pending_proj) else 2
                                if boundary:
                                    want = len(pending_proj)
                                for _ in range(min(want, len(pending_proj))):
                                    pending_proj.pop(0)()
                            if do_A and boundary and ib + 2 < NIB:
                                emit_xdma(ib + 2)
                                pending_proj = proj_closures(ib + 2)
                            emit_S(t)
                        if t >= vlag:
                            emit_V(t - vlag)
                        if fillers and fillers[0][0] <= t:
                            fillers.pop(0)[1]()
                    while fillers:
                        fillers.pop(0)[1]()
        if loop_ctx is not None:
            loop_ctx.__exit__(None, None, None)

    nc.compile()
    return nc


def make_in_maps(x, wq, bq, wk, bk, wv, bv, wo, bo):
    import ml_dtypes
    jj = np.arange(128)[:, None]
    ii = np.arange(128)[None, :]
    inv_masks = np.where(jj > ii, -1e9, 0.0).astype(np.float32)
    inv_masks_bf = inv_masks.astype(ml_dtypes.bfloat16)
    ident_bf = np.eye(128, dtype=ml_dtypes.bfloat16)

    in_maps = []
    for c in range(NCORES):
        b, g = c // G, c % G
        cs = slice(g * C, (g + 1) * C)
        in_maps.append({
            "x": np.ascontiguousarray(x[b]),
            "wq": np.ascontiguousarray(wq[:, cs]),
            "wk": np.ascontiguousarray(wk[:, cs]),
            "wv": np.ascontiguousarray(wv[:, cs]),
            "wo": np.ascontiguousarray(wo[cs, :]),
            "bq": np.ascontiguousarray(bq[cs].reshape(C, 1)),
            "bk": np.ascontiguousarray(bk[cs].reshape(C, 1)),
            "invmask_bf": inv_masks_bf,
            "ident_bf": ident_bf,
            "ident": np.eye(128, dtype=np.float32),
        })
    return in_maps


_NC_CACHE = {}


def _get_nc(mm_mode=MM_MODE):
    if mm_mode not in _NC_CACHE:
        _NC_CACHE[mm_mode] = build_nc(mm_mode)
    return _NC_CACHE[mm_mode]


def kernel(x, mask, wq, bq, wk, bk, wv, bv, wo, bo, _trace=False, _results=None):
    from concourse.bass_utils import run_bass_kernel_spmd

    x = np.asarray(x, dtype=np.float32)
    nc = _get_nc()
    in_maps = make_in_maps(x, np.asarray(wq), np.asarray(bq), np.asarray(wk),
                           np.asarray(bk), np.asarray(wv), np.asarray(bv),
                           np.asarray(wo), np.asarray(bo))
    res = run_bass_kernel_spmd(nc, in_maps, core_ids=list(range(NCORES)),
                               trace=_trace)
    if _results is not None:
        _results.append(res)
    # constant row: y += bv (since attn rows sum to 1)  =>  out += bv@wo + bo
    row_const = (np.asarray(bv, np.float64) @ np.asarray(wo, np.float64)
                 + np.asarray(bo, np.float64)).astype(np.float32)
    out = np.empty((B, T, D), dtype=np.float32)
    for b in range(B):
        out[b] = (res.results[2 * b]["out"] + res.results[2 * b + 1]["out"]
                  + row_const)
    return out
